# revision 24
# baseline (speedup 1.0000x reference)
"""4D conv (VALID, stride 1) + ReLU on 8 trn2 NeuronCores via Bass/Tile.

Problem shapes (hardcoded):
  pic_in: [B=2, C=16, D1=8, D2=8, D3=64, D4=64] f32
  weight: [O=32, I=16, 3, 3, 3, 3] f32
  out:    [B=2, O=32, 6, 6, 62, 62] f32

Strategy:
  - Shard output D3 rows across 8 cores (8 rows each, starts S_STARTS;
    neighbouring shards overlap by up to 1 row so every core runs the
    identical program shape).
  - Host pre-builds, per core, a tap-shifted stack of the input so the
    matmul contraction packs input channels with conv taps across SBUF
    partitions. Weights are pre-transposed to lhsT[(tap,c), k1, (k2,o)].
  - Each PSUM tile z[(k2,o), r, n] for one (b, d1o, d2') accumulates
    matmuls over the k1 taps (free-dim AP offsets). M = 96 packs the k2
    taps so the conv's k2 reduction becomes a shifted add across three
    PSUM tiles on the vector engine, followed by ReLU on scalar.
  - design "v1": K = 48 = C x k3-taps; 9 matmuls (k1 x k4) per z tile.
  - design "v3": K = 128 = C x 8 (k3,k4)-taps; the 9th tap (2,2) is a
    K=16 matmul reading the (2,0)-tap partitions (96:112) at a +2
    column offset. 6 matmuls per z tile. The input tile is split in
    overlapping d1 halves so DMA double-buffers against compute.
  - dtype "f32r": fp32 data, PE rounds to ~13-bit mantissa, 4x faster
    streaming than plain fp32 (1 cycle/row at N>=256).
"""

import numpy as np

import concourse.bacc as bacc
import concourse.mybir as mybir
from concourse.tile import TileContext
from concourse.bass_utils import run_bass_kernel_spmd

F32 = mybir.dt.float32
BF16 = mybir.dt.bfloat16
NP_BF16 = mybir.dt.np(mybir.dt.bfloat16)

B, C, D1, D2, D3, D4 = 2, 16, 8, 8, 64, 64
O = 32
D1o, D2o, D3o, D4o = 6, 6, 62, 62
S_STARTS = [0, 8, 15, 23, 31, 39, 46, 54]  # per-core output d3 row starts
ROWS = 8  # output d3 rows per core

TAPS14 = [(k1, k4) for k1 in range(3) for k4 in range(3)]
# v3: 8 taps stacked in partitions; (2,2) handled via offset on the (2,0) rows
TAPS8 = [(k3, k4) for k3 in range(3) for k4 in range(3)][:8]
D1_HALVES = [(0, range(0, 3)), (3, range(3, 6))]  # (d1' base, d1o range)


def _post_add(nc, zs, y_sb, d2o):
    # y[d2o] = relu(z[d2o][k2=0] + z[d2o+1][k2=1] + z[d2o+2][k2=2])
    # HW rule: at most one PSUM input per instruction.
    ys = y_sb[:, d2o]
    nc.scalar.activation(ys, zs[d2o][0:32], mybir.ActivationFunctionType.Copy)
    nc.vector.tensor_add(ys, ys, zs[d2o + 1][32:64])
    nc.vector.tensor_add(ys, ys, zs[d2o + 2][64:96])
    nc.scalar.activation(ys, ys, mybir.ActivationFunctionType.Relu)


def build_program_v5(reps: int = 1, loop_n: int = 0):
    """bf16 staged inputs; d1-halved SBUF tiles (no chunk re-DMA); PSUM
    k2-merge split across ACT/DVE/Pool/DMA; bf16 output."""
    nc = bacc.Bacc("TRN2", target_bir_lowering=False, debug=False)
    y = nc.dram_tensor("y", [B, O, D1o, D2o, ROWS, D4o], BF16, kind="ExternalOutput")
    xs = nc.dram_tensor("xs", [B, 128, D1, D2, ROWS, D4], BF16, kind="ExternalInput")
    xs2 = nc.dram_tensor("xs2", [B, 48, D1o, D2, ROWS, D4], BF16, kind="ExternalInput")
    wr = nc.dram_tensor("wr", [128, 3, 96], BF16, kind="ExternalInput")
    wr2 = nc.dram_tensor("wr2", [48, 96], BF16, kind="ExternalInput")

    ACT_COPY = mybir.ActivationFunctionType.Copy
    ACT_RELU = mybir.ActivationFunctionType.Relu
    ADD = mybir.AluOpType.add

    with TileContext(nc) as tc:
        with (
            tc.tile_pool(name="w", bufs=1) as wpool,
            tc.tile_pool(name="x", bufs=2) as xpool,
            tc.tile_pool(name="x2", bufs=2) as x2pool,
            tc.tile_pool(name="ps", bufs=4, space="PSUM") as pspool,
            tc.tile_pool(name="zs", bufs=2) as zsbpool,
            tc.tile_pool(name="zr", bufs=2) as zrpool,
            tc.tile_pool(name="t", bufs=6) as tpool,
            tc.tile_pool(name="yb", bufs=2) as ypool,
        ):
            wt = wpool.tile([128, 3, 96], BF16)
            nc.sync.dma_start(out=wt[:], in_=wr.ap())
            wt2 = wpool.tile([48, 96], BF16)
            nc.sync.dma_start(out=wt2[:], in_=wr2.ap())

            def body():
                groups = [(b, h0, dl) for b in range(B) for h0 in (0, 3)
                          for dl in range(3)]
                xtiles = {}  # b -> (xta, xtb)

                def alloc_x(b):
                    if b not in xtiles:
                        xtiles[b] = (
                            xpool.tile([128, 4, D2, ROWS, D4], BF16,
                                       tag="xta", name=f"xta{b}"),
                            xpool.tile([128, 4, D2, ROWS, D4], BF16,
                                       tag="xtb", name=f"xtb{b}"),
                        )
                    return xtiles[b]

                def load_chunk(b, d1):
                    # one d1-slice of xs into the right quarter tile
                    xta, xtb = alloc_x(b)
                    xt, di = (xta, d1) if d1 < 4 else (xtb, d1 - 4)
                    nc.sync.dma_start(
                        out=xt[:, di : di + 1], in_=xs.ap()[b, :, d1 : d1 + 1]
                    )

                x2tiles = {}

                def load_xt2(g):
                    if g >= len(groups):
                        return
                    b, h0, dl = groups[g]
                    x2tiles[g] = x2pool.tile([48, D2, ROWS, D4], BF16,
                                             tag="xt2", name=f"xt2_{g}")
                    nc.sync.dma_start(out=x2tiles[g][:], in_=xs2.ap()[b, :, h0 + dl])

                # need-ordered DMA issue plan: chunk (b, d1) keyed by group
                chunk_plan = {
                    -1: [(0, 0), (0, 1), (0, 2)],
                    0: [(0, 3)], 1: [(0, 4)], 2: [(0, 5)],
                    3: [(0, 6), (1, 0)], 4: [(0, 7), (1, 1)],
                    5: [(1, 2), (1, 3)], 6: [(1, 4), (1, 5)],
                    7: [(1, 6)], 8: [(1, 7)],
                }
                for b_, d1_ in chunk_plan[-1]:
                    load_chunk(b_, d1_)
                load_xt2(0)

                for g, (b, h0, dl) in enumerate(groups):
                    load_xt2(g + 1)
                    for b_, d1_ in chunk_plan.get(g, []):
                        load_chunk(b_, d1_)
                    d1o = h0 + dl
                    xta, xtb = xtiles[b]
                    xt2 = x2tiles.pop(g)
                    # four 2-bank PSUM supertiles, one per d2p pair
                    # (inner 8x64xf32 = exactly one 2KB bank)
                    zq = [
                        pspool.tile([96, 2, ROWS, 64], F32, tag="z", name=f"z{q}")
                        for q in range(4)
                    ]
                    # weight-stationary k1=0,1 sweeps; then per-d2p
                    # [k1=2, w2-stop] pairs so PSUM pairs finalize
                    # incrementally and the bf16 spill starts early.
                    for k1 in range(2):
                        d1 = d1o + k1
                        xt, di = (xta, d1) if d1 < 4 else (xtb, d1 - 4)
                        for d2p in range(D2):
                            nc.tensor.matmul(
                                zq[d2p // 2][:, d2p % 2, :, 0:D4o],
                                lhsT=wt[:, k1, :],
                                rhs=xt[:, di, d2p, :, 0:D4o],
                                start=(k1 == 0),
                                stop=False,
                            )
                    d1 = d1o + 2
                    xt, di = (xta, d1) if d1 < 4 else (xtb, d1 - 4)
                    zsb = zsbpool.tile([96, D2, ROWS, D4o], BF16, tag="zsb")
                    d2p_order = range(D2 - 1, -1, -1) if g == len(groups) - 1 \
                        else range(D2)
                    for d2p in d2p_order:
                        nc.tensor.matmul(
                            zq[d2p // 2][:, d2p % 2, :, 0:D4o],
                            lhsT=wt[:, 2, :],
                            rhs=xt[:, di, d2p, :, 0:D4o],
                            start=False,
                            stop=False,
                        )
                        nc.tensor.matmul(
                            zq[d2p // 2][:, d2p % 2, :, 0:D4o],
                            lhsT=wt2[:],
                            rhs=xt2[:, d2p, :, 0:D4o],
                            start=False,
                            stop=True,
                        )
                        if d2p % 2 == 1 and g != len(groups) - 1:
                            # spill the finished pair to SBUF bf16;
                            # releases the PSUM pair immediately
                            q = d2p // 2
                            src = zq[q][:, :, :, 0:D4o]
                            dst = zsb[:, 2 * q : 2 * q + 2, :, :]
                            nc.scalar.activation(dst, src, ACT_COPY)
                    if g == len(groups) - 1 or groups[g + 1][0] != b:
                        xtiles.pop(b, None)
                    # all-SBUF bf16 merge, two 3-d2o halves (half a only
                    # needs spills q0..q2, so it overlaps q3's matmuls);
                    # in place on zsb's k2=0 blocks
                    y_sb = ypool.tile([O, D2o, ROWS, D4o], BF16, tag="ysb")
                    if g == len(groups) - 1:
                        # fine per-d2o chains straight from PSUM: shortest
                        # drain tail (no spill hop). d2p ran descending, so
                        # do d2o descending too (inputs finish in that order)
                        for d2o in range(D2o - 1, -1, -1):
                            t1 = tpool.tile([O, ROWS, D4o], F32, tag="t1",
                                            name=f"t1_{d2o}")
                            j0, j1, j2 = d2o, d2o + 1, d2o + 2
                            nc.scalar.activation(
                                t1[:], zq[j0 // 2][0:32, j0 % 2, :, 0:D4o],
                                ACT_COPY,
                            )
                            nc.vector.tensor_tensor(
                                t1[:], t1[:], zq[j1 // 2][32:64, j1 % 2, :, 0:D4o],
                                op=ADD,
                            )
                            nc.vector.tensor_tensor(
                                t1[:], t1[:], zq[j2 // 2][64:96, j2 % 2, :, 0:D4o],
                                op=ADD,
                            )
                            nc.scalar.activation(y_sb[:, d2o], t1[:], ACT_RELU)
                            nc.gpsimd.dma_start(
                                out=y.ap()[b, :, d1o, d2o : d2o + 1],
                                in_=y_sb[:, d2o : d2o + 1],
                            )
                        continue
                    # half b first: its zsb inputs (banks 3..7) finish first
                    pieces = ((0, 3), (3, 3))
                    for lo, w in pieces:
                        # realign k2=1,2 blocks to partition base 0 (cheap
                        # 4x-rate bf16 SBUF copies); SBUF+SBUF adds must be
                        # base-aligned per walrus
                        k1b = zrpool.tile([32, 3, ROWS, D4o], BF16, tag="k1b",
                                          name=f"k1b{lo}")
                        nc.vector.tensor_copy(
                            k1b[:], zsb[32:64, lo + 1 : lo + 1 + w]
                        )
                        k2b = zrpool.tile([32, 3, ROWS, D4o], BF16, tag="k2b",
                                          name=f"k2b{lo}")
                        nc.vector.tensor_copy(
                            k2b[:], zsb[64:96, lo + 2 : lo + 2 + w]
                        )
                        m3 = zsb[0:32, lo : lo + w]
                        nc.vector.tensor_tensor(m3, m3, k1b[:, 0:w], op=ADD)
                        nc.vector.tensor_tensor(m3, m3, k2b[:, 0:w], op=ADD)
                        nc.gpsimd.tensor_scalar_max(y_sb[:, lo : lo + w], m3, 0.0)
                        # y writeback on the idle Pool SWDGE queue
                        nc.gpsimd.dma_start(
                            out=y.ap()[b, :, d1o, lo : lo + w],
                            in_=y_sb[:, lo : lo + w],
                        )

            if loop_n > 0:
                with tc.For_i(0, loop_n, 1):
                    body()
            else:
                for _rep in range(reps):
                    body()
    nc.compile()
    return nc


def build_program(dtype_mode: str = "f32r", reps: int = 1, loop_n: int = 0,
                  design: str = "v5", tap_outer: bool = False):
    assert design == "v5", "only the v5 design remains"
    return build_program_v5(reps=reps, loop_n=loop_n)


def make_in_maps_v5(pic_in: np.ndarray, weight: np.ndarray):
    pic_in = np.ascontiguousarray(pic_in, dtype=np.float32)
    weight = np.asarray(weight, dtype=np.float32)
    # w[o, c, k1, k2, k3, k4] -> wt_k[c, k1, (k2,o), k3, k4]
    wt_k = weight.transpose(1, 2, 3, 0, 4, 5).reshape(16, 3, 96, 3, 3)
    wre = np.zeros((128, 3, 96), np.float32)
    for t, (k3, k4) in enumerate(TAPS8):
        wre[t * 16 : (t + 1) * 16, :, :] = wt_k[:, :, :, k3, k4]
    wr2 = np.zeros((48, 96), np.float32)
    for k1 in range(3):
        wr2[k1 * 16 : (k1 + 1) * 16, :] = wt_k[:, k1, :, 2, 2]
    wre = wre.astype(NP_BF16)
    wr2 = wr2.astype(NP_BF16)

    in_maps = []
    for s in S_STARTS:
        xst = np.zeros((B, 128, D1, D2, ROWS, D4), np.float32)
        for t, (k3, k4) in enumerate(TAPS8):
            xst[:, t * 16 : (t + 1) * 16, :, :, :, : D4 - k4] = pic_in[
                :, :, :, :, s + k3 : s + k3 + ROWS, k4:
            ]
        xs2 = np.zeros((B, 48, D1o, D2, ROWS, D4), np.float32)
        for k1 in range(3):
            for d1o in range(D1o):
                xs2[:, k1 * 16 : (k1 + 1) * 16, d1o, :, :, : D4 - 2] = pic_in[
                    :, :, d1o + k1, :, s + 2 : s + 2 + ROWS, 2:
                ]
        in_maps.append(
            {
                "xs": xst.astype(NP_BF16),
                "xs2": xs2.astype(NP_BF16),
                "wr": wre,
                "wr2": wr2,
            }
        )
    return in_maps


def make_in_maps(pic_in: np.ndarray, weight: np.ndarray, design: str = "v5"):
    assert design == "v5"
    return make_in_maps_v5(pic_in, weight)


def _unused_make_in_maps_legacy(pic_in, weight, design):
    pic_in = np.ascontiguousarray(pic_in, dtype=np.float32)
    weight = np.asarray(weight, dtype=np.float32)
    in_maps = []
    if design == "v1":
        # lhsT[(k3,c), k1, k4, (k2,o)] = w[o, c, k1, k2, k3, k4]
        wre = np.ascontiguousarray(
            weight.transpose(4, 1, 2, 5, 3, 0).reshape(48, 3, 3, 96)
        )
        for s in S_STARTS:
            xst = np.empty((B, 48, D1, D2, ROWS, D4), np.float32)
            for k3 in range(3):
                xst[:, k3 * 16 : (k3 + 1) * 16] = pic_in[
                    :, :, :, :, s + k3 : s + k3 + ROWS, :
                ]
            in_maps.append({"xs": xst, "wr": wre})
        return in_maps

    # w[o, c, k1, k2, k3, k4] -> wt_k[c, k1, (k2,o), k3, k4]
    wt_k = weight.transpose(1, 2, 3, 0, 4, 5).reshape(16, 3, 96, 3, 3)

    if design == "v3":
        # slot 0 = 8 stacked taps, slot 1 = tap (2,2) on partitions 96:112
        wre = np.zeros((128, 3, 2, 96), np.float32)
        for t, (k3, k4) in enumerate(TAPS8):
            wre[t * 16 : (t + 1) * 16, :, 0, :] = wt_k[:, :, :, k3, k4]
        wre[96:112, :, 1, :] = wt_k[:, :, :, 2, 2]
    else:  # v4: slots 0..2 = per-k1 8-tap weights, slot 3 = (k1,c)-stacked (2,2)
        wre = np.zeros((128, 4, 96), np.float32)
        for t, (k3, k4) in enumerate(TAPS8):
            wre[t * 16 : (t + 1) * 16, 0:3, :] = wt_k[:, :, :, k3, k4].transpose(
                0, 1, 2
            )
        for k1 in range(3):
            wre[k1 * 16 : (k1 + 1) * 16, 3, :] = wt_k[:, k1, :, 2, 2]

    for s in S_STARTS:
        xst = np.zeros((B, 128, D1, D2, ROWS, D4), np.float32)
        for t, (k3, k4) in enumerate(TAPS8):
            xst[:, t * 16 : (t + 1) * 16, :, :, :, : D4 - k4] = pic_in[
                :, :, :, :, s + k3 : s + k3 + ROWS, k4:
            ]
        im = {"xs": xst, "wr": wre}
        if design == "v4":
            xs2 = np.zeros((B, 3, 48, 2, D2, ROWS, D4), np.float32)
            for ci in range(3):
                for k1 in range(3):
                    for dl in range(2):
                        xs2[:, ci, k1 * 16 : (k1 + 1) * 16, dl, :, :, : D4 - 2] = (
                            pic_in[:, :, 2 * ci + dl + k1, :, s + 2 : s + 2 + ROWS, 2:]
                        )
            im["xs2"] = xs2
        in_maps.append(im)
    return in_maps


def assemble_output(results):
    out = np.empty((B, O, D1o, D2o, D3o, D4o), np.float32)
    for i, s in enumerate(S_STARTS):
        out[:, :, :, :, s : s + ROWS, :] = np.asarray(results[i]["y"], np.float32)
    return out


def kernel(pic_in: np.ndarray, weight: np.ndarray) -> np.ndarray:
    nc = build_program(design="v5")
    in_maps = make_in_maps(pic_in, weight, design="v5")
    res = run_bass_kernel_spmd(nc, in_maps, list(range(8)))
    return assemble_output(res.results)



# revision 26
# speedup vs baseline: 1.1675x; 1.1675x over previous
"""4D conv (VALID, stride 1) + ReLU on 8 trn2 NeuronCores via Bass/Tile.

Problem shapes (hardcoded):
  pic_in: [B=2, C=16, D1=8, D2=8, D3=64, D4=64] f32
  weight: [O=32, I=16, 3, 3, 3, 3] f32
  out:    [B=2, O=32, 6, 6, 62, 62] f32

Design (v6):
  - Shard output D3 rows across 8 cores (8 rows each, starts S_STARTS).
  - Host pre-stacks, per core, 8 (k3,k4)-shifted copies of the input so
    the matmul contraction packs channels x taps across all 128 SBUF
    partitions (bf16; rel err ~6e-3, gate is 2e-2). Tap (2,2) is covered
    by a separate 48-partition (k1,c) stack (xs2) so each PSUM tile
    finishes in 4 matmul streams: 3x K=128 (one per k1, weight-stationary
    sweeps over d2) + 1x K=48.
  - PSUM: four 2-bank supertiles per (b,d1o) group, M=96 packs (k2,O).
    Finished pairs spill to SBUF as bf16 (ACT, 96-wide), freeing PSUM
    immediately; the k2 merge runs on spilled data: DVE realign copies
    (32-aligned partition moves), bf16 2x adds, DVE relu-max. GPSIMD is
    avoided entirely (its tensor ops and SWDGE DMAs are very slow on HW).
  - DMA: d1-sliced xs chunk loads drip-fed in a need-ordered issue plan
    on the SP queue so loads overlap compute; y writebacks ride the ACT
    HWDGE queue; bf16 halves all traffic vs f32.
  - Last group uses fine per-d2o PSUM chains and per-d2o writebacks to
    shorten the end-of-program drain tail.
"""

import numpy as np

import concourse.bacc as bacc
import concourse.mybir as mybir
from concourse.tile import TileContext
from concourse.bass_utils import run_bass_kernel_spmd

F32 = mybir.dt.float32
BF16 = mybir.dt.bfloat16
NP_BF16 = mybir.dt.np(mybir.dt.bfloat16)

B, C, D1, D2, D3, D4 = 2, 16, 8, 8, 64, 64
O = 32
D1o, D2o, D3o, D4o = 6, 6, 62, 62
S_STARTS = [0, 8, 15, 23, 31, 39, 46, 54]  # per-core output d3 row starts
ROWS = 8  # output d3 rows per core

TAPS14 = [(k1, k4) for k1 in range(3) for k4 in range(3)]
# v3: 8 taps stacked in partitions; (2,2) handled via offset on the (2,0) rows
TAPS8 = [(k3, k4) for k3 in range(3) for k4 in range(3)][:8]
D1_HALVES = [(0, range(0, 3)), (3, range(3, 6))]  # (d1' base, d1o range)


def _post_add(nc, zs, y_sb, d2o):
    # y[d2o] = relu(z[d2o][k2=0] + z[d2o+1][k2=1] + z[d2o+2][k2=2])
    # HW rule: at most one PSUM input per instruction.
    ys = y_sb[:, d2o]
    nc.scalar.activation(ys, zs[d2o][0:32], mybir.ActivationFunctionType.Copy)
    nc.vector.tensor_add(ys, ys, zs[d2o + 1][32:64])
    nc.vector.tensor_add(ys, ys, zs[d2o + 2][64:96])
    nc.scalar.activation(ys, ys, mybir.ActivationFunctionType.Relu)


def build_program_v5(reps: int = 1, loop_n: int = 0):
    """bf16 staged inputs; d1-halved SBUF tiles (no chunk re-DMA); PSUM
    k2-merge split across ACT/DVE/Pool/DMA; bf16 output."""
    nc = bacc.Bacc("TRN2", target_bir_lowering=False, debug=False)
    y = nc.dram_tensor("y", [B, O, D1o, D2o, ROWS, D4o], BF16, kind="ExternalOutput")
    xs = nc.dram_tensor("xs", [B, 128, D1, D2, ROWS, D4], BF16, kind="ExternalInput")
    xs2 = nc.dram_tensor("xs2", [B, 48, D1o, D2, ROWS, D4], BF16, kind="ExternalInput")
    wr = nc.dram_tensor("wr", [128, 3, 96], BF16, kind="ExternalInput")
    wr2 = nc.dram_tensor("wr2", [48, 96], BF16, kind="ExternalInput")

    ACT_COPY = mybir.ActivationFunctionType.Copy
    ACT_RELU = mybir.ActivationFunctionType.Relu
    ADD = mybir.AluOpType.add

    with TileContext(nc) as tc:
        with (
            tc.tile_pool(name="w", bufs=1) as wpool,
            tc.tile_pool(name="x", bufs=2) as xpool,
            tc.tile_pool(name="x2", bufs=2) as x2pool,
            tc.tile_pool(name="ps", bufs=4, space="PSUM") as pspool,
            tc.tile_pool(name="zs", bufs=2) as zsbpool,
            tc.tile_pool(name="zr", bufs=2) as zrpool,
            tc.tile_pool(name="t", bufs=6) as tpool,
            tc.tile_pool(name="yb", bufs=2) as ypool,
        ):
            wt = wpool.tile([128, 3, 96], BF16)
            nc.sync.dma_start(out=wt[:], in_=wr.ap())
            wt2 = wpool.tile([48, 96], BF16)
            nc.sync.dma_start(out=wt2[:], in_=wr2.ap())

            def body():
                groups = [(b, h0, dl) for b in range(B) for h0 in (0, 3)
                          for dl in range(3)]
                xtiles = {}  # b -> (xta, xtb)

                def alloc_x(b):
                    if b not in xtiles:
                        xtiles[b] = (
                            xpool.tile([128, 4, D2, ROWS, D4], BF16,
                                       tag="xta", name=f"xta{b}"),
                            xpool.tile([128, 4, D2, ROWS, D4], BF16,
                                       tag="xtb", name=f"xtb{b}"),
                        )
                    return xtiles[b]

                def load_chunk(b, d1):
                    # one d1-slice of xs into the right quarter tile
                    xta, xtb = alloc_x(b)
                    xt, di = (xta, d1) if d1 < 4 else (xtb, d1 - 4)
                    nc.sync.dma_start(
                        out=xt[:, di : di + 1], in_=xs.ap()[b, :, d1 : d1 + 1]
                    )

                x2tiles = {}

                def load_xt2(g):
                    if g >= len(groups):
                        return
                    b, h0, dl = groups[g]
                    x2tiles[g] = x2pool.tile([48, D2, ROWS, D4], BF16,
                                             tag="xt2", name=f"xt2_{g}")
                    nc.sync.dma_start(out=x2tiles[g][:], in_=xs2.ap()[b, :, h0 + dl])

                # need-ordered DMA issue plan: chunk (b, d1) keyed by group
                chunk_plan = {
                    -1: [(0, 0), (0, 1), (0, 2)],
                    0: [(0, 3)], 1: [(0, 4)], 2: [(0, 5)],
                    3: [(0, 6), (1, 0)], 4: [(0, 7), (1, 1)],
                    5: [(1, 2), (1, 3)], 6: [(1, 4), (1, 5)],
                    7: [(1, 6)], 8: [(1, 7)],
                }
                for b_, d1_ in chunk_plan[-1]:
                    load_chunk(b_, d1_)
                load_xt2(0)

                for g, (b, h0, dl) in enumerate(groups):
                    load_xt2(g + 1)
                    for b_, d1_ in chunk_plan.get(g, []):
                        load_chunk(b_, d1_)
                    d1o = h0 + dl
                    xta, xtb = xtiles[b]
                    xt2 = x2tiles.pop(g)
                    # four 2-bank PSUM supertiles, one per d2p pair
                    # (inner 8x64xf32 = exactly one 2KB bank)
                    zq = [
                        pspool.tile([96, 2, ROWS, 64], F32, tag="z", name=f"z{q}")
                        for q in range(4)
                    ]
                    # weight-stationary k1=0,1 sweeps; then per-d2p
                    # [k1=2, w2-stop] pairs so PSUM pairs finalize
                    # incrementally and the bf16 spill starts early.
                    for k1 in range(2):
                        d1 = d1o + k1
                        xt, di = (xta, d1) if d1 < 4 else (xtb, d1 - 4)
                        for d2p in range(D2):
                            nc.tensor.matmul(
                                zq[d2p // 2][:, d2p % 2, :, 0:D4o],
                                lhsT=wt[:, k1, :],
                                rhs=xt[:, di, d2p, :, 0:D4o],
                                start=(k1 == 0),
                                stop=False,
                            )
                    d1 = d1o + 2
                    xt, di = (xta, d1) if d1 < 4 else (xtb, d1 - 4)
                    zsb = zsbpool.tile([96, D2, ROWS, D4o], BF16, tag="zsb")
                    d2p_order = range(D2 - 1, -1, -1) if g == len(groups) - 1 \
                        else range(D2)
                    for d2p in d2p_order:
                        nc.tensor.matmul(
                            zq[d2p // 2][:, d2p % 2, :, 0:D4o],
                            lhsT=wt[:, 2, :],
                            rhs=xt[:, di, d2p, :, 0:D4o],
                            start=False,
                            stop=False,
                        )
                        nc.tensor.matmul(
                            zq[d2p // 2][:, d2p % 2, :, 0:D4o],
                            lhsT=wt2[:],
                            rhs=xt2[:, d2p, :, 0:D4o],
                            start=False,
                            stop=True,
                        )
                        if d2p % 2 == 1 and g != len(groups) - 1:
                            # spill the finished pair to SBUF bf16;
                            # releases the PSUM pair immediately
                            q = d2p // 2
                            src = zq[q][:, :, :, 0:D4o]
                            dst = zsb[:, 2 * q : 2 * q + 2, :, :]
                            nc.scalar.activation(dst, src, ACT_COPY)
                    if g == len(groups) - 1 or groups[g + 1][0] != b:
                        xtiles.pop(b, None)
                    # all-SBUF bf16 merge, two 3-d2o halves (half a only
                    # needs spills q0..q2, so it overlaps q3's matmuls);
                    # in place on zsb's k2=0 blocks
                    y_sb = ypool.tile([O, D2o, ROWS, D4o], BF16, tag="ysb")
                    if g == len(groups) - 1:
                        # fine per-d2o chains straight from PSUM: shortest
                        # drain tail (no spill hop). d2p ran descending, so
                        # do d2o descending too (inputs finish in that order)
                        for d2o in range(D2o - 1, -1, -1):
                            t1 = tpool.tile([O, ROWS, D4o], F32, tag="t1",
                                            name=f"t1_{d2o}")
                            j0, j1, j2 = d2o, d2o + 1, d2o + 2
                            nc.scalar.activation(
                                t1[:], zq[j0 // 2][0:32, j0 % 2, :, 0:D4o],
                                ACT_COPY,
                            )
                            nc.vector.tensor_tensor(
                                t1[:], t1[:], zq[j1 // 2][32:64, j1 % 2, :, 0:D4o],
                                op=ADD,
                            )
                            nc.vector.tensor_tensor(
                                t1[:], t1[:], zq[j2 // 2][64:96, j2 % 2, :, 0:D4o],
                                op=ADD,
                            )
                            nc.scalar.activation(y_sb[:, d2o], t1[:], ACT_RELU)
                            nc.scalar.dma_start(
                                out=y.ap()[b, :, d1o, d2o : d2o + 1],
                                in_=y_sb[:, d2o : d2o + 1],
                            )
                        continue
                    # half b first: its zsb inputs (banks 3..7) finish first
                    pieces = ((0, 3), (3, 3))
                    for lo, w in pieces:
                        # realign k2=1,2 blocks to partition base 0 (cheap
                        # 4x-rate bf16 SBUF copies); SBUF+SBUF adds must be
                        # base-aligned per walrus
                        k1b = zrpool.tile([32, 3, ROWS, D4o], BF16, tag="k1b",
                                          name=f"k1b{lo}")
                        nc.vector.tensor_copy(
                            k1b[:], zsb[32:64, lo + 1 : lo + 1 + w]
                        )
                        k2b = zrpool.tile([32, 3, ROWS, D4o], BF16, tag="k2b",
                                          name=f"k2b{lo}")
                        nc.vector.tensor_copy(
                            k2b[:], zsb[64:96, lo + 2 : lo + 2 + w]
                        )
                        m3 = zsb[0:32, lo : lo + w]
                        nc.vector.tensor_tensor(m3, m3, k1b[:, 0:w], op=ADD)
                        nc.vector.tensor_tensor(m3, m3, k2b[:, 0:w], op=ADD)
                        nc.vector.tensor_scalar_max(y_sb[:, lo : lo + w], m3, 0.0)
                    # single y writeback per group on the ACT HWDGE queue
                    nc.scalar.dma_start(out=y.ap()[b, :, d1o], in_=y_sb[:])

            if loop_n > 0:
                with tc.For_i(0, loop_n, 1):
                    body()
            else:
                for _rep in range(reps):
                    body()
    nc.compile()
    return nc


def build_program(dtype_mode: str = "f32r", reps: int = 1, loop_n: int = 0,
                  design: str = "v5", tap_outer: bool = False):
    assert design == "v5", "only the v5 design remains"
    return build_program_v5(reps=reps, loop_n=loop_n)


def make_in_maps_v5(pic_in: np.ndarray, weight: np.ndarray):
    pic_in = np.ascontiguousarray(pic_in, dtype=np.float32)
    weight = np.asarray(weight, dtype=np.float32)
    # w[o, c, k1, k2, k3, k4] -> wt_k[c, k1, (k2,o), k3, k4]
    wt_k = weight.transpose(1, 2, 3, 0, 4, 5).reshape(16, 3, 96, 3, 3)
    wre = np.zeros((128, 3, 96), np.float32)
    for t, (k3, k4) in enumerate(TAPS8):
        wre[t * 16 : (t + 1) * 16, :, :] = wt_k[:, :, :, k3, k4]
    wr2 = np.zeros((48, 96), np.float32)
    for k1 in range(3):
        wr2[k1 * 16 : (k1 + 1) * 16, :] = wt_k[:, k1, :, 2, 2]
    wre = wre.astype(NP_BF16)
    wr2 = wr2.astype(NP_BF16)

    in_maps = []
    for s in S_STARTS:
        xst = np.zeros((B, 128, D1, D2, ROWS, D4), np.float32)
        for t, (k3, k4) in enumerate(TAPS8):
            xst[:, t * 16 : (t + 1) * 16, :, :, :, : D4 - k4] = pic_in[
                :, :, :, :, s + k3 : s + k3 + ROWS, k4:
            ]
        xs2 = np.zeros((B, 48, D1o, D2, ROWS, D4), np.float32)
        for k1 in range(3):
            for d1o in range(D1o):
                xs2[:, k1 * 16 : (k1 + 1) * 16, d1o, :, :, : D4 - 2] = pic_in[
                    :, :, d1o + k1, :, s + 2 : s + 2 + ROWS, 2:
                ]
        in_maps.append(
            {
                "xs": xst.astype(NP_BF16),
                "xs2": xs2.astype(NP_BF16),
                "wr": wre,
                "wr2": wr2,
            }
        )
    return in_maps


def make_in_maps(pic_in: np.ndarray, weight: np.ndarray, design: str = "v5"):
    assert design == "v5"
    return make_in_maps_v5(pic_in, weight)


def _unused_make_in_maps_legacy(pic_in, weight, design):
    pic_in = np.ascontiguousarray(pic_in, dtype=np.float32)
    weight = np.asarray(weight, dtype=np.float32)
    in_maps = []
    if design == "v1":
        # lhsT[(k3,c), k1, k4, (k2,o)] = w[o, c, k1, k2, k3, k4]
        wre = np.ascontiguousarray(
            weight.transpose(4, 1, 2, 5, 3, 0).reshape(48, 3, 3, 96)
        )
        for s in S_STARTS:
            xst = np.empty((B, 48, D1, D2, ROWS, D4), np.float32)
            for k3 in range(3):
                xst[:, k3 * 16 : (k3 + 1) * 16] = pic_in[
                    :, :, :, :, s + k3 : s + k3 + ROWS, :
                ]
            in_maps.append({"xs": xst, "wr": wre})
        return in_maps

    # w[o, c, k1, k2, k3, k4] -> wt_k[c, k1, (k2,o), k3, k4]
    wt_k = weight.transpose(1, 2, 3, 0, 4, 5).reshape(16, 3, 96, 3, 3)

    if design == "v3":
        # slot 0 = 8 stacked taps, slot 1 = tap (2,2) on partitions 96:112
        wre = np.zeros((128, 3, 2, 96), np.float32)
        for t, (k3, k4) in enumerate(TAPS8):
            wre[t * 16 : (t + 1) * 16, :, 0, :] = wt_k[:, :, :, k3, k4]
        wre[96:112, :, 1, :] = wt_k[:, :, :, 2, 2]
    else:  # v4: slots 0..2 = per-k1 8-tap weights, slot 3 = (k1,c)-stacked (2,2)
        wre = np.zeros((128, 4, 96), np.float32)
        for t, (k3, k4) in enumerate(TAPS8):
            wre[t * 16 : (t + 1) * 16, 0:3, :] = wt_k[:, :, :, k3, k4].transpose(
                0, 1, 2
            )
        for k1 in range(3):
            wre[k1 * 16 : (k1 + 1) * 16, 3, :] = wt_k[:, k1, :, 2, 2]

    for s in S_STARTS:
        xst = np.zeros((B, 128, D1, D2, ROWS, D4), np.float32)
        for t, (k3, k4) in enumerate(TAPS8):
            xst[:, t * 16 : (t + 1) * 16, :, :, :, : D4 - k4] = pic_in[
                :, :, :, :, s + k3 : s + k3 + ROWS, k4:
            ]
        im = {"xs": xst, "wr": wre}
        if design == "v4":
            xs2 = np.zeros((B, 3, 48, 2, D2, ROWS, D4), np.float32)
            for ci in range(3):
                for k1 in range(3):
                    for dl in range(2):
                        xs2[:, ci, k1 * 16 : (k1 + 1) * 16, dl, :, :, : D4 - 2] = (
                            pic_in[:, :, 2 * ci + dl + k1, :, s + 2 : s + 2 + ROWS, 2:]
                        )
            im["xs2"] = xs2
        in_maps.append(im)
    return in_maps


def assemble_output(results):
    out = np.empty((B, O, D1o, D2o, D3o, D4o), np.float32)
    for i, s in enumerate(S_STARTS):
        out[:, :, :, :, s : s + ROWS, :] = np.asarray(results[i]["y"], np.float32)
    return out


def kernel(pic_in: np.ndarray, weight: np.ndarray) -> np.ndarray:
    nc = build_program(design="v5")
    in_maps = make_in_maps(pic_in, weight, design="v5")
    res = run_bass_kernel_spmd(nc, in_maps, list(range(8)))
    return assemble_output(res.results)

